# revision 2
# baseline (speedup 1.0000x reference)
"""GCN TRN2 kernel entry point (dev version; final will be self-contained)."""
import os
import numpy as np

LAST_EXEC_TIME_NS = None
LAST_RESULT = None


def _install_ntff_shim():
    """Provide antenv.axon_hooks (missing in this image) so trace=True works."""
    import sys, types
    try:
        from antenv.axon_hooks import get_axon_ntff_profile_hook  # noqa
        return  # already present
    except ImportError:
        pass
    m = types.ModuleType("antenv.axon_hooks")
    _h = [None]
    m.set_axon_ntff_profile_hook = lambda h: _h.__setitem__(0, h)
    m.get_axon_ntff_profile_hook = lambda: _h[0]
    sys.modules["antenv.axon_hooks"] = m
    try:
        import antenv
        antenv.axon_hooks = m
    except ImportError:
        pass
    try:
        from trn_agent_boot.trn_boot import _ntff_profile_via_ctypes
        hook = _ntff_profile_via_ctypes("/opt/axon/libaxon_pjrt.so")
        m.set_axon_ntff_profile_hook(hook)
    except Exception as e:
        print(f"ntff shim: hook unavailable: {e}")


def kernel(features, edge_index, W1, b1, W2, b2, Wm1, bm1, Wm2, bm2):
    global LAST_EXEC_TIME_NS, LAST_RESULT
    import sys
    sys.path.insert(0, os.path.dirname(os.path.abspath(__file__)))
    from gcn_kernel import Config, prep_host, build_program, make_in_maps, unshard
    import concourse.bass_utils as bass_utils

    trace = os.environ.get("GCN_TRACE", "1") == "1"
    if trace:
        _install_ntff_shim()
        # offline sandbox: skip artifact upload
        bass_utils.upload_artifacts = lambda tmpdir: str(tmpdir)

    cfg = Config(N=features.shape[0], E=edge_index.shape[1],
                 table_bf16=os.environ.get("GCN_BF16", "0") == "1")
    weights = dict(W1=W1, b1=b1, W2=W2, b2=b2, Wm1=Wm1, bm1=bm1, Wm2=Wm2, bm2=bm2)
    host = prep_host(cfg, features, edge_index)
    nc = build_program(cfg, host, weights)
    in_maps = make_in_maps(cfg, host, weights)
    tmpdir = os.environ.get("GCN_TMPDIR") or None
    r = bass_utils.run_bass_kernel_spmd(nc, in_maps, list(range(cfg.n_cores)),
                                        trace=trace, tmpdir=tmpdir)
    LAST_EXEC_TIME_NS = r.exec_time_ns
    LAST_RESULT = r
    return unshard(cfg, r.results)
